# revision 7
# baseline (speedup 1.0000x reference)
"""Capsule-routing kernel (einsum bni,nkdi,nk->bkd + squash) on 8 trn2 cores.

Sharding: over the contraction axis n (2048 -> 256 per core). Each core
reads only its slice of x (8 MB) and W (8 MB) -- every input byte is read
exactly once machine-wide. Each core emits a partial s[b,(k,d)] over its
n-slice; the host sums the 8 partials (4 MB total) and applies the tiny
squash nonlinearity (131K elements).

Per-core device program:
  - DMA x slice  [B=256, n_l*i=4096]  naturally (contiguous per partition)
  - DMA W slice  [n_l=256, k*d*i=8192] naturally
  - softmax weights Rs[n,k] multiplied into W with per-partition scalars
    (partition = n, one op per k-slice)
  - PE-transposes of x (input AP strided over n at fixed i) to put the
    contraction dim n on partitions: xT[(i)] tiles [n=128, B=128]
  - 32 accumulating fp32 matmuls per B-half into PSUM [B=128, k*d=512]
"""

import os
import sys

import numpy as np

if "/opt/trn_rl_repo" not in sys.path:
    sys.path.insert(0, "/opt/trn_rl_repo")

import bass_rust as _bass_rust
import concourse.bass as bass
import concourse.mybir as mybir
from concourse.bass_utils import run_bass_kernel_spmd
from concourse.masks import make_identity
from concourse.tile import TileContext

# The walrus build in this container accepts at most ONE sync-wait per
# instruction; Tile's kernel-tail drain carries one wait per outstanding
# semaphore (12 here) and fails codegen.  Split it into a chain of
# single-wait drains on the sync sequencer (program order makes the chain
# equivalent to the original multi-wait drain).
if not getattr(TileContext, "_split_drain_patched", False):

    def _split_drain_and_barrier(self, tick_clock, wait_clock):
        gc = tick_clock.global_clock
        vals = list(gc)
        for j, v in enumerate(vals):
            if v > 0:
                sub = [0] * len(vals)
                sub[j] = v
                d = self.nc.sync.drain()
                wait_clock.add_sem_waits(
                    d.ins,
                    _bass_rust.ScopedClock({None: _bass_rust.VectorClock(sub)}),
                )
        self.nc.all_engine_barrier()
        assert self.sems is not None
        popped = self.nc._tile_sem_poison_stack.pop()
        assert popped is self._sem_poison
        self.nc.clear_and_free_semaphores(list(self.sems.allocated().values()))

    TileContext._drain_and_barrier = _split_drain_and_barrier
    TileContext._split_drain_patched = True

NCORES = 8
B, N, I = 256, 2048, 16
K, D = 32, 16
NL = N // NCORES  # 256 n-values per core
KD = K * D  # 512
F_W = K * D * I  # 8192
F_X = NL * I  # 4096
EPS = 1e-7

FP32 = mybir.dt.float32


def build_bass() -> bass.Bass:
    nc = bass.Bass()
    x_d = nc.dram_tensor("xs", [B, F_X], FP32, kind="ExternalInput")
    w_d = nc.dram_tensor("ws", [NL, F_W], FP32, kind="ExternalInput")
    r_d = nc.dram_tensor("rs", [NL, K], FP32, kind="ExternalInput")
    o_d = nc.dram_tensor("out", [B, KD], FP32, kind="ExternalOutput")

    with TileContext(nc) as tc:
        with (
            tc.tile_pool(name="big", bufs=1) as big,
            tc.tile_pool(name="ps_t", bufs=3, space="PSUM") as ps_t,
            tc.tile_pool(name="ps_warm", bufs=1, space="PSUM") as ps_warm,
            tc.tile_pool(name="ps_acc", bufs=1, space="PSUM") as ps_acc,
        ):
            ident = big.tile([128, 128], FP32, tag="ident")
            make_identity(nc, ident)

            # input DMAs
            rs_in, ws, xs = [], [], []
            for t in range(2):
                r_sb = big.tile([128, K], FP32, tag=f"rs{t}")
                nc.sync.dma_start(out=r_sb[:], in_=r_d[t * 128 : (t + 1) * 128, :])
                rs_in.append(r_sb)
            for t in range(2):
                w_sb = big.tile([128, F_W], FP32, tag=f"w{t}")
                nc.sync.dma_start(out=w_sb[:], in_=w_d[t * 128 : (t + 1) * 128, :])
                ws.append(w_sb)
            for h in range(2):
                x_sb = big.tile([128, F_X], FP32, tag=f"x{h}")
                nc.sync.dma_start(out=x_sb[:], in_=x_d[h * 128 : (h + 1) * 128, :])
                xs.append(x_sb)

            # The HW instruction structs accept a single sync-wait, so no
            # instruction may carry two unabsorbed cross-engine deps.  Absorb
            # each producer into the consuming engine's program order up
            # front: gpsimd ident + x DMAs into PE (cheap transposes), rs + W
            # DMAs into DVE (tiny copies).
            rs = []
            with tc.high_priority():
                warm_ps = ps_warm.tile([128, 512], FP32, tag="identwarm")
                nc.tensor.transpose(warm_ps[:, 0:128], ident[:], ident[:])
                for h in range(2):
                    nc.tensor.transpose(
                        warm_ps[:, 128 * (1 + h) : 128 * (2 + h)],
                        xs[h][:, 0:128],
                        ident[:],
                    )
                for t in range(2):
                    r_c = big.tile([128, K], FP32, tag=f"rsc{t}")
                    nc.vector.tensor_copy(r_c[:], rs_in[t][:])
                    rs.append(r_c)
                for t in range(2):
                    w_t = big.tile([128, 1], FP32, tag=f"wtouch{t}")
                    nc.vector.tensor_copy(w_t[:], ws[t][:, 0:1])

            # scale W by Rs[n, k]: per-partition scalar, one op per k slice
            for t in range(2):
                for k in range(K):
                    sl = ws[t][:, k * (D * I) : (k + 1) * (D * I)]
                    nc.vector.tensor_scalar_mul(sl, sl, rs[t][:, k : k + 1])

            # transpose x: [B=128, n(stride 16) at fixed i] -> [n=128, B=128]
            xt = {}
            for h in range(2):
                x_v = xs[h].rearrange("p (n i) -> p n i", i=I)
                for t in range(2):
                    for i in range(I):
                        pst = ps_t.tile([128, 128], FP32)
                        nc.tensor.transpose(
                            pst[:], x_v[:, t * 128 : (t + 1) * 128, i], ident[:]
                        )
                        sb = big.tile([128, 128], FP32, tag=f"xt{h}_{t}_{i}")
                        nc.vector.tensor_copy(sb[:], pst[:])
                        xt[(h, t, i)] = sb

            # main matmuls: acc[b, (k d)] += xT[n, b].T @ Wr[n, (k d) @ i]
            for h in range(2):
                acc = ps_acc.tile([128, KD], FP32, tag=f"acc{h}")
                idx = 0
                for t in range(2):
                    w_v = ws[t].rearrange("p (k d i) -> p k d i", d=D, i=I)
                    for i in range(I):
                        nc.tensor.matmul(
                            acc[:],
                            xt[(h, t, i)][:],
                            w_v[:, :, :, i],
                            start=(idx == 0),
                            stop=(idx == 31),
                        )
                        idx += 1
                o_sb = big.tile([128, KD], FP32, tag=f"o{h}")
                nc.scalar.copy(o_sb[:], acc[:])
                nc.sync.dma_start(
                    out=o_d[h * 128 : (h + 1) * 128, :], in_=o_sb[:]
                )

    return nc


_CACHE: dict = {}

# test.py sets these for profiling; harness never touches them.
LAST_RESULTS = None


def _trace_kwargs():
    if os.environ.get("BASS_KERNEL_TRACE") == "1":
        cores = os.environ.get("BASS_KERNEL_TRACE_CORES", "0")
        return dict(trace=True, trace_cores=[int(c) for c in cores.split(",")])
    return {}


def kernel(x: np.ndarray, W: np.ndarray, R: np.ndarray) -> np.ndarray:
    global LAST_RESULTS
    x = np.asarray(x, dtype=np.float32)
    W = np.asarray(W, dtype=np.float32)
    R = np.asarray(R, dtype=np.float32)

    # softmax over n (65K elements -- host)
    Rm = R.max(axis=0, keepdims=True)
    e = np.exp(R - Rm)
    Rs = (e / e.sum(axis=0, keepdims=True)).astype(np.float32)

    in_maps = []
    for c in range(NCORES):
        sl = slice(c * NL, (c + 1) * NL)
        in_maps.append(
            {
                "xs": np.ascontiguousarray(x[:, sl, :]).reshape(B, F_X),
                "ws": np.ascontiguousarray(W[sl]).reshape(NL, F_W),
                "rs": np.ascontiguousarray(Rs[sl]),
            }
        )

    if "nc" not in _CACHE:
        _CACHE["nc"] = build_bass()
    nc = _CACHE["nc"]

    res = run_bass_kernel_spmd(
        nc, in_maps, core_ids=list(range(NCORES)), **_trace_kwargs()
    )
    LAST_RESULTS = res

    s = np.zeros((B, KD), np.float32)
    for r in res.results:
        s += r["out"]
    s = s.reshape(B, K, D)
    sq = np.sum(np.square(s), axis=-1, keepdims=True) + EPS
    v = (np.sqrt(sq) / (1.0 + sq)) * s
    return v.astype(np.float32)


if __name__ == "__main__":
    rng = np.random.default_rng(0)
    x = rng.standard_normal((B, N, I), dtype=np.float32)
    W = (rng.standard_normal((N, K, D, I), dtype=np.float32) * 0.05).astype(np.float32)
    R = rng.standard_normal((N, K), dtype=np.float32)
    out = kernel(x, W, R)
    print("out", out.shape, out.dtype, float(np.abs(out).mean()))


# revision 11
# speedup vs baseline: 1.3991x; 1.3991x over previous
"""Capsule-routing kernel (einsum bni,nkdi,nk->bkd + squash) on 8 trn2 cores.

Sharding: over the contraction axis n (2048 -> 256 per core). Each core
reads only its slice of x (8 MB) and W (8 MB) -- every input byte is read
exactly once machine-wide. Each core emits a partial s[b,(k,d)] over its
n-slice; the host sums the 8 partials (4 MB total) and applies the tiny
squash nonlinearity (131K elements).

Per-core device program:
  - DMA x slice  [B=256, n_l*i=4096]  naturally (contiguous per partition)
  - DMA W slice  [n_l=256, k*d*i=8192] naturally
  - softmax weights Rs[n,k] multiplied into W with per-partition scalars
    (partition = n, one op per k-slice)
  - PE-transposes of x (input AP strided over n at fixed i) to put the
    contraction dim n on partitions: xT[(i)] tiles [n=128, B=128]
  - 32 accumulating fp32 matmuls per B-half into PSUM [B=128, k*d=512]
"""

import os
import sys

import numpy as np

if "/opt/trn_rl_repo" not in sys.path:
    sys.path.insert(0, "/opt/trn_rl_repo")

import bass_rust as _bass_rust
import concourse.bass as bass
import concourse.mybir as mybir
from concourse.bass_utils import run_bass_kernel_spmd
from concourse.masks import make_identity
from concourse.tile import TileContext

# The walrus build in this container accepts at most ONE sync-wait per
# instruction; Tile's kernel-tail drain carries one wait per outstanding
# semaphore (12 here) and fails codegen.  Split it into a chain of
# single-wait drains on the sync sequencer (program order makes the chain
# equivalent to the original multi-wait drain).
if not getattr(TileContext, "_split_drain_patched", False):

    def _split_drain_and_barrier(self, tick_clock, wait_clock):
        gc = tick_clock.global_clock
        vals = list(gc)
        for j, v in enumerate(vals):
            if v > 0:
                sub = [0] * len(vals)
                sub[j] = v
                d = self.nc.sync.drain()
                wait_clock.add_sem_waits(
                    d.ins,
                    _bass_rust.ScopedClock({None: _bass_rust.VectorClock(sub)}),
                )
        self.nc.all_engine_barrier()
        assert self.sems is not None
        popped = self.nc._tile_sem_poison_stack.pop()
        assert popped is self._sem_poison
        self.nc.clear_and_free_semaphores(list(self.sems.allocated().values()))

    TileContext._drain_and_barrier = _split_drain_and_barrier
    TileContext._split_drain_patched = True

NCORES = 8
B, N, I = 256, 2048, 16
K, D = 32, 16
NL = N // NCORES  # 256 n-values per core
KD = K * D  # 512
F_W = K * D * I  # 8192
F_X = NL * I  # 4096
EPS = 1e-7

FP32 = mybir.dt.float32
BF16 = mybir.dt.bfloat16


def build_bass() -> bass.Bass:
    nc = bass.Bass()
    x_d = nc.dram_tensor("xs", [B, F_X], FP32, kind="ExternalInput")
    w_d = nc.dram_tensor("ws", [NL, F_W], FP32, kind="ExternalInput")
    r_d = nc.dram_tensor("rs", [NL, K], FP32, kind="ExternalInput")
    o_d = nc.dram_tensor("out", [B, KD], FP32, kind="ExternalOutput")

    with TileContext(nc) as tc:
        with (
            tc.tile_pool(name="big", bufs=1) as big,
            tc.tile_pool(name="ps_t", bufs=3, space="PSUM") as ps_t,
            tc.tile_pool(name="ps_warm", bufs=1, space="PSUM") as ps_warm,
            tc.tile_pool(name="ps_acc", bufs=1, space="PSUM") as ps_acc,
        ):
            ident = big.tile([128, 128], FP32, tag="ident")
            make_identity(nc, ident)

            # input DMAs
            rs_in, ws, xs = [], [], []
            for t in range(2):
                r_sb = big.tile([128, K], FP32, tag=f"rs{t}")
                nc.sync.dma_start(out=r_sb[:], in_=r_d[t * 128 : (t + 1) * 128, :])
                rs_in.append(r_sb)
            for t in range(2):
                w_sb = big.tile([128, F_W], FP32, tag=f"w{t}")
                nc.sync.dma_start(out=w_sb[:], in_=w_d[t * 128 : (t + 1) * 128, :])
                ws.append(w_sb)
            for h in range(2):
                x_sb = big.tile([128, F_X], FP32, tag=f"x{h}")
                nc.sync.dma_start(out=x_sb[:], in_=x_d[h * 128 : (h + 1) * 128, :])
                xs.append(x_sb)

            # The HW instruction structs accept a single sync-wait, so no
            # instruction may carry two unabsorbed cross-engine deps.  Absorb
            # each producer into the consuming engine's program order up
            # front: gpsimd ident + x DMAs into PE (cheap transposes), rs + W
            # DMAs into DVE (tiny copies).
            rs = []
            with tc.high_priority():
                warm_ps = ps_warm.tile([128, 512], FP32, tag="identwarm")
                nc.tensor.transpose(warm_ps[:, 0:128], ident[:], ident[:])
                for h in range(2):
                    nc.tensor.transpose(
                        warm_ps[:, 128 * (1 + h) : 128 * (2 + h)],
                        xs[h][:, 0:128],
                        ident[:],
                    )
                for t in range(2):
                    r_c = big.tile([128, K], FP32, tag=f"rsc{t}")
                    nc.vector.tensor_copy(r_c[:], rs_in[t][:])
                    rs.append(r_c)
                for t in range(2):
                    w_t = big.tile([128, 1], FP32, tag=f"wtouch{t}")
                    nc.vector.tensor_copy(w_t[:], ws[t][:, 0:1])

            # scale W by Rs[n, k] (per-partition scalar) and convert to bf16
            # for the PE: one op per k slice, fp32 in -> bf16 out
            wb = []
            for t in range(2):
                w_b = big.tile([128, F_W], BF16, tag=f"wb{t}")
                wb.append(w_b)
            for t in range(2):
                for k in range(K):
                    sl_in = ws[t][:, k * (D * I) : (k + 1) * (D * I)]
                    sl_out = wb[t][:, k * (D * I) : (k + 1) * (D * I)]
                    nc.vector.tensor_scalar_mul(sl_out, sl_in, rs[t][:, k : k + 1])

            # transpose x: [B=128, n(stride 16) at fixed i] -> [n=128, B=128]
            xt = {}
            for h in range(2):
                x_v = xs[h].rearrange("p (n i) -> p n i", i=I)
                for t in range(2):
                    for i in range(I):
                        pst = ps_t.tile([128, 128], FP32)
                        nc.tensor.transpose(
                            pst[:], x_v[:, t * 128 : (t + 1) * 128, i], ident[:]
                        )
                        sb = big.tile([128, 128], BF16, tag=f"xt{h}_{t}_{i}")
                        nc.vector.tensor_copy(sb[:], pst[:])
                        xt[(h, t, i)] = sb

            # main matmuls: acc[b, (k d)] += xT[n, b].T @ Wr[n, (k d) @ i]
            for h in range(2):
                acc = ps_acc.tile([128, KD], FP32, tag=f"acc{h}")
                idx = 0
                for t in range(2):
                    w_v = wb[t].rearrange("p (k d i) -> p k d i", d=D, i=I)
                    for i in range(I):
                        nc.tensor.matmul(
                            acc[:],
                            xt[(h, t, i)][:],
                            w_v[:, :, :, i],
                            start=(idx == 0),
                            stop=(idx == 31),
                        )
                        idx += 1
                o_sb = big.tile([128, KD], FP32, tag=f"o{h}")
                nc.scalar.copy(o_sb[:], acc[:])
                nc.sync.dma_start(
                    out=o_d[h * 128 : (h + 1) * 128, :], in_=o_sb[:]
                )

    return nc


_CACHE: dict = {}

# test.py sets these for profiling; harness never touches them.
LAST_RESULTS = None


def _trace_kwargs():
    if os.environ.get("BASS_KERNEL_TRACE") == "1":
        cores = os.environ.get("BASS_KERNEL_TRACE_CORES", "0")
        return dict(trace=True, trace_cores=[int(c) for c in cores.split(",")])
    return {}


def kernel(x: np.ndarray, W: np.ndarray, R: np.ndarray) -> np.ndarray:
    global LAST_RESULTS
    x = np.asarray(x, dtype=np.float32)
    W = np.asarray(W, dtype=np.float32)
    R = np.asarray(R, dtype=np.float32)

    # softmax over n (65K elements -- host)
    Rm = R.max(axis=0, keepdims=True)
    e = np.exp(R - Rm)
    Rs = (e / e.sum(axis=0, keepdims=True)).astype(np.float32)

    in_maps = []
    for c in range(NCORES):
        sl = slice(c * NL, (c + 1) * NL)
        in_maps.append(
            {
                "xs": np.ascontiguousarray(x[:, sl, :]).reshape(B, F_X),
                "ws": np.ascontiguousarray(W[sl]).reshape(NL, F_W),
                "rs": np.ascontiguousarray(Rs[sl]),
            }
        )

    if "nc" not in _CACHE:
        _CACHE["nc"] = build_bass()
    nc = _CACHE["nc"]

    res = run_bass_kernel_spmd(
        nc, in_maps, core_ids=list(range(NCORES)), **_trace_kwargs()
    )
    LAST_RESULTS = res

    s = np.zeros((B, KD), np.float32)
    for r in res.results:
        s += r["out"]
    s = s.reshape(B, K, D)
    sq = np.sum(np.square(s), axis=-1, keepdims=True) + EPS
    v = (np.sqrt(sq) / (1.0 + sq)) * s
    return v.astype(np.float32)


if __name__ == "__main__":
    rng = np.random.default_rng(0)
    x = rng.standard_normal((B, N, I), dtype=np.float32)
    W = (rng.standard_normal((N, K, D, I), dtype=np.float32) * 0.05).astype(np.float32)
    R = rng.standard_normal((N, K), dtype=np.float32)
    out = kernel(x, W, R)
    print("out", out.shape, out.dtype, float(np.abs(out).mean()))


# revision 19
# speedup vs baseline: 2.1127x; 1.5100x over previous
"""Capsule-routing kernel (einsum bni,nkdi,nk->bkd + squash) on 8 trn2 cores.

Sharding: over the contraction axis n (2048 -> 256 per core). Each core
reads only its slice of x (8 MB) and W (8 MB) -- every input byte is read
exactly once machine-wide. Each core emits a partial s[b,(k,d)] over its
n-slice; the host sums the 8 partials (4 MB total) and applies the tiny
squash nonlinearity (131K elements).

Per-core device program:
  - DMA x slice  [B=256, n_l*i=4096]  naturally (contiguous per partition)
  - DMA W slice  [n_l=256, k*d*i=8192] naturally
  - softmax weights Rs[n,k] multiplied into W with per-partition scalars
    (partition = n, one op per k-slice)
  - PE-transposes of x (input AP strided over n at fixed i) to put the
    contraction dim n on partitions: xT[(i)] tiles [n=128, B=128]
  - 32 accumulating fp32 matmuls per B-half into PSUM [B=128, k*d=512]
"""

import os
import sys

import numpy as np

if "/opt/trn_rl_repo" not in sys.path:
    sys.path.insert(0, "/opt/trn_rl_repo")

import bass_rust as _bass_rust
import concourse.bass as bass
import concourse.mybir as mybir
from concourse.bass_utils import run_bass_kernel_spmd
from concourse.masks import make_identity
from concourse.tile import TileContext

# The walrus build in this container accepts at most ONE sync-wait per
# instruction; Tile's kernel-tail drain carries one wait per outstanding
# semaphore (12 here) and fails codegen.  Split it into a chain of
# single-wait drains on the sync sequencer (program order makes the chain
# equivalent to the original multi-wait drain).
if not getattr(TileContext, "_split_drain_patched", False):

    def _split_drain_and_barrier(self, tick_clock, wait_clock):
        gc = tick_clock.global_clock
        vals = list(gc)
        for j, v in enumerate(vals):
            if v > 0:
                sub = [0] * len(vals)
                sub[j] = v
                d = self.nc.sync.drain()
                wait_clock.add_sem_waits(
                    d.ins,
                    _bass_rust.ScopedClock({None: _bass_rust.VectorClock(sub)}),
                )
        self.nc.all_engine_barrier()
        assert self.sems is not None
        popped = self.nc._tile_sem_poison_stack.pop()
        assert popped is self._sem_poison
        self.nc.clear_and_free_semaphores(list(self.sems.allocated().values()))

    TileContext._drain_and_barrier = _split_drain_and_barrier
    TileContext._split_drain_patched = True

NCORES = 8
B, N, I = 256, 2048, 16
K, D = 32, 16
NL = N // NCORES  # 256 n-values per core
KD = K * D  # 512
F_W = K * D * I  # 8192
F_X = NL * I  # 4096
EPS = 1e-7

FP32 = mybir.dt.float32
BF16 = mybir.dt.bfloat16


def build_bass() -> bass.Bass:
    nc = bass.Bass()
    x_d = nc.dram_tensor("xs", [B, F_X], FP32, kind="ExternalInput")
    w_d = nc.dram_tensor("ws", [NL, F_W], FP32, kind="ExternalInput")
    r_d = nc.dram_tensor("rs", [NL, K], FP32, kind="ExternalInput")
    o_d = nc.dram_tensor("out", [B, KD], FP32, kind="ExternalOutput")

    with TileContext(nc) as tc:
        with (
            tc.tile_pool(name="big", bufs=1) as big,
            tc.tile_pool(name="ps_t", bufs=3, space="PSUM") as ps_t,
            tc.tile_pool(name="ps_warm", bufs=1, space="PSUM") as ps_warm,
            tc.tile_pool(name="ps_acc", bufs=1, space="PSUM") as ps_acc,
        ):
            ident = big.tile([128, 128], FP32, tag="ident")
            make_identity(nc, ident)

            # input DMAs.  W is chunked (2 MB per chunk) so scaling overlaps
            # the stream.  Total DMA count (inputs + the one output) is kept
            # at 8 = the number of DMAHW lanes, so no lane is reused and no
            # DMA needs a lane-serialization wait.
            WCH = 2  # chunks per W tile
            KPC = K // WCH  # k values per chunk
            ws, xs = [], []
            rs_all = big.tile([128, 2 * K], FP32, tag="rs_all")
            nc.sync.dma_start(
                out=rs_all[:], in_=r_d.rearrange("(t p) k -> p t k", t=2)
            )
            for h in range(2):
                x_sb = big.tile([128, F_X], FP32, tag=f"x{h}")
                nc.sync.dma_start(out=x_sb[:], in_=x_d[h * 128 : (h + 1) * 128, :])
                xs.append(x_sb)
            CW = F_W // WCH
            for t in range(2):
                w_sb = big.tile([128, F_W], FP32, tag=f"w{t}")
                for c in range(WCH):
                    nc.sync.dma_start(
                        out=w_sb[:, c * CW : (c + 1) * CW],
                        in_=w_d[t * 128 : (t + 1) * 128, c * CW : (c + 1) * CW],
                    )
                ws.append(w_sb)

            # The HW instruction structs accept a single sync-wait, so no
            # instruction may carry two unabsorbed cross-engine deps.  Absorb
            # each producer into the consuming engine's program order up
            # front: gpsimd ident + x DMAs into PE (cheap transposes), rs + W
            # DMAs into DVE (tiny copies).
            rs = []
            with tc.high_priority():
                warm_ps = ps_warm.tile([128, 512], FP32, tag="identwarm")
                nc.tensor.transpose(warm_ps[:, 0:128], ident[:], ident[:])
                for h in range(2):
                    nc.tensor.transpose(
                        warm_ps[:, 128 * (1 + h) : 128 * (2 + h)],
                        xs[h][:, 0:128],
                        ident[:],
                    )
                for t in range(2):
                    r_c = big.tile([128, K], FP32, tag=f"rsc{t}")
                    nc.vector.tensor_copy(r_c[:], rs_all[:, t * K : (t + 1) * K])
                    rs.append(r_c)
                for t in range(2):
                    for c in range(WCH):
                        w_t = big.tile([128, 1], FP32, tag=f"wtouch{t}_{c}")
                        nc.vector.tensor_copy(w_t[:], ws[t][:, c * CW : c * CW + 1])

            # scale W by Rs[n, k], convert to bf16, store i-major ((i k d)
            # layout) so the matmul moving operand is contiguous.  One
            # broadcast tensor_tensor per (tile, chunk): rs operand uses
            # step-0 dims over (d, i); the write side eats the stride.
            wb = []
            for t in range(2):
                w_b = big.tile([128, F_W], BF16, tag=f"wb{t}")
                wb.append(w_b)
            # wb free index = i*(K*D) + k*D + d
            wb_v = [w_b.rearrange("p (i k d) -> p i k d", k=K, d=D) for w_b in wb]
            ws_v = [w.rearrange("p (k d i) -> p k d i", d=D, i=I) for w in ws]
            for c in range(WCH):
                k0, k1 = c * KPC, (c + 1) * KPC
                for t in range(2):
                    # iterate (i, k, d): the bf16 write side is then fully
                    # contiguous; the strided cost lands on the fp32 read
                    sl_in = ws_v[t][:, k0:k1, :, :].rearrange("p k d i -> p i k d")
                    sl_out = wb_v[t][:, :, k0:k1, :]
                    r_sl = rs[t][:, k0:k1]
                    r_b = bass.AP(
                        tensor=r_sl.tensor,
                        offset=r_sl.offset,
                        ap=[r_sl.ap[0], [0, I], r_sl.ap[1], [0, D]],
                    )
                    nc.vector.tensor_mul(sl_out, sl_in, r_b)

            # transpose x: [B=128, n(stride 16) at fixed i] -> [n=128, B=128]
            xt = {}
            for h in range(2):
                x_v = xs[h].rearrange("p (n i) -> p n i", i=I)
                for t in range(2):
                    for i in range(I):
                        pst = ps_t.tile([128, 128], FP32)
                        nc.tensor.transpose(
                            pst[:], x_v[:, t * 128 : (t + 1) * 128, i], ident[:]
                        )
                        sb = big.tile([128, 128], BF16, tag=f"xt{h}_{t}_{i}")
                        nc.vector.tensor_copy(sb[:], pst[:])
                        xt[(h, t, i)] = sb

            # main matmuls: acc_h[b, (k d)] += xT[n, b].T @ Wr[n, (k d)] per i.
            # B-halves interleaved so both accumulators finish together.
            accs = [
                ps_acc.tile([128, KD], FP32, tag=f"acc{h}", name=f"acc{h}")
                for h in range(2)
            ]
            idx = 0
            for t in range(2):
                for i in range(I):
                    rhs = wb[t][:, i * KD : (i + 1) * KD]
                    for h in range(2):
                        nc.tensor.matmul(
                            accs[h][:],
                            xt[(h, t, i)][:],
                            rhs,
                            start=(idx == 0),
                            stop=(idx == 31),
                        )
                    idx += 1
            o_sb = big.tile([128, 2 * KD], FP32, tag="osb")
            for h in range(2):
                nc.scalar.copy(o_sb[:, h * KD : (h + 1) * KD], accs[h][:])
            nc.sync.dma_start(
                out=o_d.rearrange("(h p) f -> p h f", h=2), in_=o_sb[:]
            )

    return nc


_CACHE: dict = {}

# test.py sets these for profiling; harness never touches them.
LAST_RESULTS = None


def _trace_kwargs():
    if os.environ.get("BASS_KERNEL_TRACE") == "1":
        cores = os.environ.get("BASS_KERNEL_TRACE_CORES", "0")
        return dict(trace=True, trace_cores=[int(c) for c in cores.split(",")])
    return {}


def kernel(x: np.ndarray, W: np.ndarray, R: np.ndarray) -> np.ndarray:
    global LAST_RESULTS
    x = np.asarray(x, dtype=np.float32)
    W = np.asarray(W, dtype=np.float32)
    R = np.asarray(R, dtype=np.float32)

    # softmax over n (65K elements -- host)
    Rm = R.max(axis=0, keepdims=True)
    e = np.exp(R - Rm)
    Rs = (e / e.sum(axis=0, keepdims=True)).astype(np.float32)

    in_maps = []
    for c in range(NCORES):
        sl = slice(c * NL, (c + 1) * NL)
        in_maps.append(
            {
                "xs": np.ascontiguousarray(x[:, sl, :]).reshape(B, F_X),
                "ws": np.ascontiguousarray(W[sl]).reshape(NL, F_W),
                "rs": np.ascontiguousarray(Rs[sl]),
            }
        )

    if "nc" not in _CACHE:
        _CACHE["nc"] = build_bass()
    nc = _CACHE["nc"]

    res = run_bass_kernel_spmd(
        nc, in_maps, core_ids=list(range(NCORES)), **_trace_kwargs()
    )
    LAST_RESULTS = res

    s = np.zeros((B, KD), np.float32)
    for r in res.results:
        s += r["out"]
    s = s.reshape(B, K, D)
    sq = np.sum(np.square(s), axis=-1, keepdims=True) + EPS
    v = (np.sqrt(sq) / (1.0 + sq)) * s
    return v.astype(np.float32)


if __name__ == "__main__":
    rng = np.random.default_rng(0)
    x = rng.standard_normal((B, N, I), dtype=np.float32)
    W = (rng.standard_normal((N, K, D, I), dtype=np.float32) * 0.05).astype(np.float32)
    R = rng.standard_normal((N, K), dtype=np.float32)
    out = kernel(x, W, R)
    print("out", out.shape, out.dtype, float(np.abs(out).mean()))


# revision 21
# speedup vs baseline: 2.6257x; 1.2428x over previous
"""Capsule-routing kernel (einsum bni,nkdi,nk->bkd + squash) on 8 trn2 cores.

Sharding: over the contraction axis n (2048 -> 256 per core). Each core
reads only its slice of x (8 MB) and W (8 MB) -- every input byte is read
exactly once machine-wide. Each core emits a partial s[b,(k,d)] over its
n-slice; the host sums the 8 partials (4 MB total) and applies the tiny
squash nonlinearity (131K elements).

Per-core device program:
  - DMA x slice  [B=256, n_l*i=4096]  naturally (contiguous per partition)
  - DMA W slice  [n_l=256, k*d*i=8192] naturally
  - softmax weights Rs[n,k] multiplied into W with per-partition scalars
    (partition = n, one op per k-slice)
  - PE-transposes of x (input AP strided over n at fixed i) to put the
    contraction dim n on partitions: xT[(i)] tiles [n=128, B=128]
  - 32 accumulating fp32 matmuls per B-half into PSUM [B=128, k*d=512]
"""

import os
import sys

import numpy as np

if "/opt/trn_rl_repo" not in sys.path:
    sys.path.insert(0, "/opt/trn_rl_repo")

import bass_rust as _bass_rust
import concourse.bass as bass
import concourse.mybir as mybir
from concourse.bass_utils import run_bass_kernel_spmd
from concourse.masks import make_identity
from concourse.tile import TileContext

# The walrus build in this container accepts at most ONE sync-wait per
# instruction; Tile's kernel-tail drain carries one wait per outstanding
# semaphore (12 here) and fails codegen.  Split it into a chain of
# single-wait drains on the sync sequencer (program order makes the chain
# equivalent to the original multi-wait drain).
if not getattr(TileContext, "_split_drain_patched", False):

    def _split_drain_and_barrier(self, tick_clock, wait_clock):
        gc = tick_clock.global_clock
        vals = list(gc)
        for j, v in enumerate(vals):
            if v > 0:
                sub = [0] * len(vals)
                sub[j] = v
                d = self.nc.sync.drain()
                wait_clock.add_sem_waits(
                    d.ins,
                    _bass_rust.ScopedClock({None: _bass_rust.VectorClock(sub)}),
                )
        self.nc.all_engine_barrier()
        assert self.sems is not None
        popped = self.nc._tile_sem_poison_stack.pop()
        assert popped is self._sem_poison
        self.nc.clear_and_free_semaphores(list(self.sems.allocated().values()))

    TileContext._drain_and_barrier = _split_drain_and_barrier
    TileContext._split_drain_patched = True

NCORES = 8
B, N, I = 256, 2048, 16
K, D = 32, 16
NL = N // NCORES  # 256 n-values per core
KD = K * D  # 512
F_W = K * D * I  # 8192
F_X = NL * I  # 4096
EPS = 1e-7

FP32 = mybir.dt.float32
BF16 = mybir.dt.bfloat16


def build_bass() -> bass.Bass:
    nc = bass.Bass()
    x_d = nc.dram_tensor("xs", [B, F_X], FP32, kind="ExternalInput")
    w_d = nc.dram_tensor("ws", [NL, F_W], FP32, kind="ExternalInput")
    r_d = nc.dram_tensor("rs", [NL, K], FP32, kind="ExternalInput")
    o_d = nc.dram_tensor("out", [B, KD], FP32, kind="ExternalOutput")

    with TileContext(nc) as tc:
        with (
            tc.tile_pool(name="big", bufs=1) as big,
            tc.tile_pool(name="ps_t", bufs=3, space="PSUM") as ps_t,
            tc.tile_pool(name="ps_warm", bufs=1, space="PSUM") as ps_warm,
            tc.tile_pool(name="ps_acc", bufs=1, space="PSUM") as ps_acc,
        ):
            ident = big.tile([128, 128], FP32, tag="ident")
            make_identity(nc, ident)

            # input DMAs.  W is chunked (2 MB per chunk) so scaling overlaps
            # the stream.  Total DMA count (inputs + the one output) is kept
            # at 8 = the number of DMAHW lanes, so no lane is reused and no
            # DMA needs a lane-serialization wait.
            WCH = 2  # chunks per W tile
            KPC = K // WCH  # k values per chunk
            ws, xs = [], []
            rs_all = big.tile([128, 2 * K], FP32, tag="rs_all")
            nc.sync.dma_start(
                out=rs_all[:], in_=r_d.rearrange("(t p) k -> p t k", t=2)
            )
            for h in range(2):
                x_sb = big.tile([128, F_X], FP32, tag=f"x{h}")
                nc.sync.dma_start(out=x_sb[:], in_=x_d[h * 128 : (h + 1) * 128, :])
                xs.append(x_sb)
            CW = F_W // WCH
            for t in range(2):
                w_sb = big.tile([128, F_W], FP32, tag=f"w{t}")
                for c in range(WCH):
                    nc.sync.dma_start(
                        out=w_sb[:, c * CW : (c + 1) * CW],
                        in_=w_d[t * 128 : (t + 1) * 128, c * CW : (c + 1) * CW],
                    )
                ws.append(w_sb)

            # The HW instruction structs accept a single sync-wait, so no
            # instruction may carry two unabsorbed cross-engine deps.  Absorb
            # each producer into the consuming engine's program order up
            # front: gpsimd ident + x DMAs into PE (cheap transposes), rs + W
            # DMAs into DVE (tiny copies).
            rs = []
            with tc.high_priority():
                warm_ps = ps_warm.tile([128, 512], FP32, tag="identwarm")
                nc.tensor.transpose(warm_ps[:, 0:128], ident[:], ident[:])
                for h in range(2):
                    nc.tensor.transpose(
                        warm_ps[:, 128 * (1 + h) : 128 * (2 + h)],
                        xs[h][:, 0:128],
                        ident[:],
                    )
                for t in range(2):
                    r_c = big.tile([128, K], FP32, tag=f"rsc{t}")
                    nc.vector.tensor_copy(r_c[:], rs_all[:, t * K : (t + 1) * K])
                    rs.append(r_c)
                for t in range(2):
                    for c in range(WCH):
                        w_t = big.tile([128, 1], FP32, tag=f"wtouch{t}_{c}")
                        nc.vector.tensor_copy(w_t[:], ws[t][:, c * CW : c * CW + 1])

            # scale W by Rs[n, k] and convert to bf16.  W arrives from the
            # host already i-major ([n, i, k, d]) so both sides stream
            # contiguously; the rs operand broadcasts over (i, d) via
            # step-0 AP dims.  One op per DMA chunk (chunk = block of i).
            IPC = I // WCH  # i values per chunk
            wb = []
            for t in range(2):
                w_b = big.tile([128, F_W], BF16, tag=f"wb{t}")
                wb.append(w_b)
            for c in range(WCH):
                for t in range(2):
                    sl_in = ws[t][:, c * CW : (c + 1) * CW].rearrange(
                        "p (i k d) -> p i k d", k=K, d=D
                    )
                    sl_out = wb[t][:, c * CW : (c + 1) * CW].rearrange(
                        "p (i k d) -> p i k d", k=K, d=D
                    )
                    r_sl = rs[t]
                    r_b = bass.AP(
                        tensor=r_sl.tensor,
                        offset=r_sl.offset,
                        ap=[r_sl.ap[0], [0, IPC], r_sl.ap[1], [0, D]],
                    )
                    nc.vector.tensor_mul(sl_out, sl_in, r_b)

            # transpose x: [B=128, n(stride 16) at fixed i] -> [n=128, B=128]
            xt = {}
            for h in range(2):
                x_v = xs[h].rearrange("p (n i) -> p n i", i=I)
                for t in range(2):
                    for i in range(I):
                        pst = ps_t.tile([128, 128], FP32)
                        nc.tensor.transpose(
                            pst[:], x_v[:, t * 128 : (t + 1) * 128, i], ident[:]
                        )
                        sb = big.tile([128, 128], BF16, tag=f"xt{h}_{t}_{i}")
                        nc.vector.tensor_copy(sb[:], pst[:])
                        xt[(h, t, i)] = sb

            # main matmuls: acc_h[b, (k d)] += xT[n, b].T @ Wr[n, (k d)] per i.
            # B-halves interleaved so both accumulators finish together.
            accs = [
                ps_acc.tile([128, KD], FP32, tag=f"acc{h}", name=f"acc{h}")
                for h in range(2)
            ]
            idx = 0
            for t in range(2):
                for i in range(I):
                    rhs = wb[t][:, i * KD : (i + 1) * KD]
                    for h in range(2):
                        nc.tensor.matmul(
                            accs[h][:],
                            xt[(h, t, i)][:],
                            rhs,
                            start=(idx == 0),
                            stop=(idx == 31),
                        )
                    idx += 1
            o_sb = big.tile([128, 2 * KD], FP32, tag="osb")
            for h in range(2):
                nc.scalar.copy(o_sb[:, h * KD : (h + 1) * KD], accs[h][:])
            nc.sync.dma_start(
                out=o_d.rearrange("(h p) f -> p h f", h=2), in_=o_sb[:]
            )

    return nc


_CACHE: dict = {}

# test.py sets these for profiling; harness never touches them.
LAST_RESULTS = None


def _trace_kwargs():
    if os.environ.get("BASS_KERNEL_TRACE") == "1":
        cores = os.environ.get("BASS_KERNEL_TRACE_CORES", "0")
        return dict(trace=True, trace_cores=[int(c) for c in cores.split(",")])
    return {}


def kernel(x: np.ndarray, W: np.ndarray, R: np.ndarray) -> np.ndarray:
    global LAST_RESULTS
    x = np.asarray(x, dtype=np.float32)
    W = np.asarray(W, dtype=np.float32)
    R = np.asarray(R, dtype=np.float32)

    # softmax over n (65K elements -- host)
    Rm = R.max(axis=0, keepdims=True)
    e = np.exp(R - Rm)
    Rs = (e / e.sum(axis=0, keepdims=True)).astype(np.float32)

    # W is uploaded i-major ([n, i, k, d]) so the on-device scale + matmul
    # moving operand both stream contiguously.
    Wp = np.ascontiguousarray(W.transpose(0, 3, 1, 2)).reshape(N, F_W)
    in_maps = []
    for c in range(NCORES):
        sl = slice(c * NL, (c + 1) * NL)
        in_maps.append(
            {
                "xs": np.ascontiguousarray(x[:, sl, :]).reshape(B, F_X),
                "ws": Wp[c * NL : (c + 1) * NL],
                "rs": np.ascontiguousarray(Rs[sl]),
            }
        )

    if "nc" not in _CACHE:
        _CACHE["nc"] = build_bass()
    nc = _CACHE["nc"]

    res = run_bass_kernel_spmd(
        nc, in_maps, core_ids=list(range(NCORES)), **_trace_kwargs()
    )
    LAST_RESULTS = res

    s = np.zeros((B, KD), np.float32)
    for r in res.results:
        s += r["out"]
    s = s.reshape(B, K, D)
    sq = np.sum(np.square(s), axis=-1, keepdims=True) + EPS
    v = (np.sqrt(sq) / (1.0 + sq)) * s
    return v.astype(np.float32)


if __name__ == "__main__":
    rng = np.random.default_rng(0)
    x = rng.standard_normal((B, N, I), dtype=np.float32)
    W = (rng.standard_normal((N, K, D, I), dtype=np.float32) * 0.05).astype(np.float32)
    R = rng.standard_normal((N, K), dtype=np.float32)
    out = kernel(x, W, R)
    print("out", out.shape, out.dtype, float(np.abs(out).mean()))


# revision 26
# speedup vs baseline: 2.8600x; 1.0892x over previous
"""Capsule-routing kernel (einsum bni,nkdi,nk->bkd + squash) on 8 trn2 cores.

Sharding: over the contraction axis n (2048 -> 256 per core). Each core
reads only its slice of x (8 MB) and W (8 MB) -- every input byte is read
exactly once machine-wide. Each core emits a partial s[b,(k,d)] over its
n-slice; the host sums the 8 partials (4 MB total) and applies the tiny
squash nonlinearity (131K elements).

Per-core device program:
  - DMA x slice  [B=256, n_l*i=4096]  naturally (contiguous per partition)
  - DMA W slice  [n_l=256, k*d*i=8192] naturally
  - softmax weights Rs[n,k] multiplied into W with per-partition scalars
    (partition = n, one op per k-slice)
  - PE-transposes of x (input AP strided over n at fixed i) to put the
    contraction dim n on partitions: xT[(i)] tiles [n=128, B=128]
  - 32 accumulating fp32 matmuls per B-half into PSUM [B=128, k*d=512]
"""

import os
import sys

import numpy as np

if "/opt/trn_rl_repo" not in sys.path:
    sys.path.insert(0, "/opt/trn_rl_repo")

import bass_rust as _bass_rust
import concourse.bass as bass
import concourse.mybir as mybir
from concourse.bass_utils import run_bass_kernel_spmd
from concourse.masks import make_identity
from concourse.tile import TileContext

# The walrus build in this container accepts at most ONE sync-wait per
# instruction; Tile's kernel-tail drain carries one wait per outstanding
# semaphore (12 here) and fails codegen.  Split it into a chain of
# single-wait drains on the sync sequencer (program order makes the chain
# equivalent to the original multi-wait drain).
if not getattr(TileContext, "_split_drain_patched", False):

    def _split_drain_and_barrier(self, tick_clock, wait_clock):
        gc = tick_clock.global_clock
        vals = list(gc)
        for j, v in enumerate(vals):
            if v > 0:
                sub = [0] * len(vals)
                sub[j] = v
                d = self.nc.sync.drain()
                wait_clock.add_sem_waits(
                    d.ins,
                    _bass_rust.ScopedClock({None: _bass_rust.VectorClock(sub)}),
                )
        self.nc.all_engine_barrier()
        assert self.sems is not None
        popped = self.nc._tile_sem_poison_stack.pop()
        assert popped is self._sem_poison
        self.nc.clear_and_free_semaphores(list(self.sems.allocated().values()))

    TileContext._drain_and_barrier = _split_drain_and_barrier
    TileContext._split_drain_patched = True

NCORES = 8
B, N, I = 256, 2048, 16
K, D = 32, 16
NL = N // NCORES  # 256 n-values per core
KD = K * D  # 512
F_W = K * D * I  # 8192
F_X = NL * I  # 4096
EPS = 1e-7

FP32 = mybir.dt.float32
BF16 = mybir.dt.bfloat16


def build_bass() -> bass.Bass:
    nc = bass.Bass()
    x_d = nc.dram_tensor("xs", [B, F_X], FP32, kind="ExternalInput")
    w_d = nc.dram_tensor("ws", [NL, F_W], FP32, kind="ExternalInput")
    r_d = nc.dram_tensor("rs", [NL, K], FP32, kind="ExternalInput")
    o_d = nc.dram_tensor("out", [B, KD], FP32, kind="ExternalOutput")

    with TileContext(nc) as tc:
        with (
            tc.tile_pool(name="big", bufs=1) as big,
            tc.tile_pool(name="ps_t", bufs=3, space="PSUM") as ps_t,
            tc.tile_pool(name="ps_warm", bufs=1, space="PSUM") as ps_warm,
            tc.tile_pool(name="ps_warm2", bufs=1, space="PSUM") as ps_warm2,
            tc.tile_pool(name="ps_acc", bufs=1, space="PSUM") as ps_acc,
        ):
            ident = big.tile([128, 128], FP32, tag="ident")
            make_identity(nc, ident)

            # input DMAs, chunked (~1 MB) so compute overlaps the stream.
            # HWDGE lanes are reused (round-robin over 8); that only costs a
            # single-wait lane serialization on the DMAs themselves.  The
            # output goes out via SWDGE (gpsimd) so it never carries a
            # lane wait on top of its data dependency.
            WCH = 4  # chunks per W tile
            XCH = 2  # chunks per x tile
            ws, xs = [], []
            rs_all = big.tile([128, 2 * K], FP32, tag="rs_all")
            nc.sync.dma_start(
                out=rs_all[:], in_=r_d.rearrange("(t p) k -> p t k", t=2)
            )
            CX = F_X // XCH
            for h in range(2):
                x_sb = big.tile([128, F_X], FP32, tag=f"x{h}")
                for c in range(XCH):
                    nc.sync.dma_start(
                        out=x_sb[:, c * CX : (c + 1) * CX],
                        in_=x_d[h * 128 : (h + 1) * 128, c * CX : (c + 1) * CX],
                    )
                xs.append(x_sb)
            CW = F_W // WCH
            for t in range(2):
                w_sb = big.tile([128, F_W], FP32, tag=f"w{t}")
                for c in range(WCH):
                    nc.sync.dma_start(
                        out=w_sb[:, c * CW : (c + 1) * CW],
                        in_=w_d[t * 128 : (t + 1) * 128, c * CW : (c + 1) * CW],
                    )
                ws.append(w_sb)

            # The HW instruction structs accept a single sync-wait, so no
            # instruction may carry two unabsorbed cross-engine deps.  Absorb
            # each producer into the consuming engine's program order up
            # front: gpsimd ident + x DMAs into PE (cheap transposes), rs + W
            # DMAs into DVE (tiny copies).
            rs = []
            warm_ps = ps_warm.tile([128, 512], FP32, tag="identwarm")
            warm2_ps = ps_warm2.tile([128, 512], FP32, tag="warm2")
            with tc.high_priority():
                # PE absorbers: ident (gpsimd) + the four x chunks (DMA).
                # Warm banks are write-only, so same-bank WAW stays sem-free.
                nc.tensor.transpose(warm_ps[:, 0:128], ident[:], ident[:])
                widx = 1
                for h in range(2):
                    for c in range(XCH):
                        dst = (
                            warm_ps[:, widx * 128 : (widx + 1) * 128]
                            if widx < 4
                            else warm2_ps[:, 0:128]
                        )
                        nc.tensor.transpose(
                            dst, xs[h][:, c * CX : c * CX + 128], ident[:]
                        )
                        widx += 1
                # DVE absorbers: rs DMA + the eight W chunks.
                for t in range(2):
                    r_c = big.tile([128, K], FP32, tag=f"rsc{t}")
                    nc.vector.tensor_copy(r_c[:], rs_all[:, t * K : (t + 1) * K])
                    rs.append(r_c)
                # bf16 identity for the wb absorber transposes
                identb = big.tile([128, 128], BF16, tag="identb")
                nc.vector.tensor_copy(identb[:], ident[:])
                for t in range(2):
                    for c in range(WCH):
                        w_t = big.tile([128, 1], FP32, tag=f"wtouch{t}_{c}")
                        nc.vector.tensor_copy(w_t[:], ws[t][:, c * CW : c * CW + 1])

            # scale W by Rs[n, k] and convert to bf16.  W arrives from the
            # host already i-major ([n, i, k, d]) so both sides stream
            # contiguously; the rs operand broadcasts over (i, d) via
            # step-0 AP dims.  One op per DMA chunk (chunk = block of i).
            IPC = I // WCH  # i values per chunk
            wb = []
            for t in range(2):
                w_b = big.tile([128, F_W], BF16, tag=f"wb{t}")
                wb.append(w_b)
            for c in range(WCH):
                for t in range(2):
                    sl_in = ws[t][:, c * CW : (c + 1) * CW].rearrange(
                        "p (i k d) -> p i k d", k=K, d=D
                    )
                    sl_out = wb[t][:, c * CW : (c + 1) * CW].rearrange(
                        "p (i k d) -> p i k d", k=K, d=D
                    )
                    r_sl = rs[t]
                    r_b = bass.AP(
                        tensor=r_sl.tensor,
                        offset=r_sl.offset,
                        ap=[r_sl.ap[0], [0, IPC], r_sl.ap[1], [0, D]],
                    )
                    nc.vector.tensor_mul(sl_out, sl_in, r_b)

            # PE absorbers for the scaled W tiles: a dummy bf16 transpose per
            # (t, chunk) whose only cross-engine dep is the DVE scale above.
            # After it, matmuls reading wb[t] carry no extra DVE wait.
            warm2b_ps = ps_warm2.tile([128, 128], BF16, tag="warm2b")
            for t in range(2):
                for c in range(WCH):
                    nc.tensor.transpose(
                        warm2b_ps[:],
                        wb[t][:, c * CW : c * CW + 128],
                        identb[:],
                    )

            # transpose x: [B=128, n(stride 16) at fixed i] -> [n=128, B=128].
            # PSUM -> SBUF casts ride the otherwise-idle scalar engine.
            xt = {}
            for h in range(2):
                x_v = xs[h].rearrange("p (n i) -> p n i", i=I)
                for t in range(2):
                    for i in range(I):
                        pst = ps_t.tile([128, 128], FP32)
                        nc.tensor.transpose(
                            pst[:], x_v[:, t * 128 : (t + 1) * 128, i], ident[:]
                        )
                        sb = big.tile([128, 128], BF16, tag=f"xt{h}_{t}_{i}")
                        nc.scalar.copy(sb[:], pst[:])
                        xt[(h, t, i)] = sb

            # main matmuls: acc_h[b, (k d)] += xT[n, b].T @ Wr[n, (k d)] per i.
            # B-halves interleaved so both accumulators finish together.
            accs = [
                ps_acc.tile([128, KD], FP32, tag=f"acc{h}", name=f"acc{h}")
                for h in range(2)
            ]
            idx = 0
            for t in range(2):
                for i in range(I):
                    rhs = wb[t][:, i * KD : (i + 1) * KD]
                    for h in range(2):
                        nc.tensor.matmul(
                            accs[h][:],
                            xt[(h, t, i)][:],
                            rhs,
                            start=(idx == 0),
                            stop=(idx == 31),
                        )
                    idx += 1
            o_sb = big.tile([128, 2 * KD], FP32, tag="osb")
            for h in range(2):
                nc.scalar.copy(o_sb[:, h * KD : (h + 1) * KD], accs[h][:])
            # SWDGE so the output never waits on HWDGE lane reuse
            nc.gpsimd.dma_start(
                out=o_d.rearrange("(h p) f -> p h f", h=2), in_=o_sb[:]
            )

    return nc


_CACHE: dict = {}

# test.py sets these for profiling; harness never touches them.
LAST_RESULTS = None


def _trace_kwargs():
    if os.environ.get("BASS_KERNEL_TRACE") == "1":
        cores = os.environ.get("BASS_KERNEL_TRACE_CORES", "0")
        return dict(trace=True, trace_cores=[int(c) for c in cores.split(",")])
    return {}


def kernel(x: np.ndarray, W: np.ndarray, R: np.ndarray) -> np.ndarray:
    global LAST_RESULTS
    x = np.asarray(x, dtype=np.float32)
    W = np.asarray(W, dtype=np.float32)
    R = np.asarray(R, dtype=np.float32)

    # softmax over n (65K elements -- host)
    Rm = R.max(axis=0, keepdims=True)
    e = np.exp(R - Rm)
    Rs = (e / e.sum(axis=0, keepdims=True)).astype(np.float32)

    # W is uploaded i-major ([n, i, k, d]) so the on-device scale + matmul
    # moving operand both stream contiguously.
    Wp = np.ascontiguousarray(W.transpose(0, 3, 1, 2)).reshape(N, F_W)
    in_maps = []
    for c in range(NCORES):
        sl = slice(c * NL, (c + 1) * NL)
        in_maps.append(
            {
                "xs": np.ascontiguousarray(x[:, sl, :]).reshape(B, F_X),
                "ws": Wp[c * NL : (c + 1) * NL],
                "rs": np.ascontiguousarray(Rs[sl]),
            }
        )

    if "nc" not in _CACHE:
        _CACHE["nc"] = build_bass()
    nc = _CACHE["nc"]

    res = run_bass_kernel_spmd(
        nc, in_maps, core_ids=list(range(NCORES)), **_trace_kwargs()
    )
    LAST_RESULTS = res

    s = np.zeros((B, KD), np.float32)
    for r in res.results:
        s += r["out"]
    s = s.reshape(B, K, D)
    sq = np.sum(np.square(s), axis=-1, keepdims=True) + EPS
    v = (np.sqrt(sq) / (1.0 + sq)) * s
    return v.astype(np.float32)


if __name__ == "__main__":
    rng = np.random.default_rng(0)
    x = rng.standard_normal((B, N, I), dtype=np.float32)
    W = (rng.standard_normal((N, K, D, I), dtype=np.float32) * 0.05).astype(np.float32)
    R = rng.standard_normal((N, K), dtype=np.float32)
    out = kernel(x, W, R)
    print("out", out.shape, out.dtype, float(np.abs(out).mean()))
